# revision 1
# baseline (speedup 1.0000x reference)
"""Complex multi-head attention on 8 Trainium2 NeuronCores (Bass/Tile).

Sharding: core c -> batch b = c//4, head group hg = c%4 (4 of 16 heads).
Each core computes QKV projections for its heads, per-head complex-magnitude
softmax attention, and a partial output projection; the host sums the 4
partial outputs per batch element.

Device-side layouts (per head h):
  QT2(h) = [q_r^T(h); q_i^T(h)]  [128, N]   (built by one block-diagonal
  KT2(h) = [k_r^T(h); k_i^T(h)]  [128, N]    matmul group contracting over
  QN(h)  = [q_i^T(h); -q_r^T(h)] [128, N]    [x_real^T; x_imag^T])
  S_r^T tile = KT2.T @ QT2 slices (single K=128 fp32r matmuls)
  S_i^T tile = KT2.T @ QN slices
  m2 = S_r^2 + S_i^2 (custom DVE ops), w_unnorm = exp(exp(0.5*ln(m2)+ln(S)))
  (ln/exp share one ACT table set; softmax max-subtraction is unnecessary)
  vcat(h) = [vr|vi] (even h) / [vi|vr] (odd h) -> psumO = [or;oi] / [oi;or]
  denominator via all-ones lhsT matmul (result replicated on all partitions)
"""

import math
import os
import sys

import numpy as np

sys.path.insert(0, "/opt/trn_rl_repo")

import concourse.bass as bass
import concourse.mybir as mybir
import concourse.tile as tile
import concourse.dve_ops as dve_ops
from concourse import bacc
from concourse.bass_utils import run_bass_kernel_spmd
from concourse.dve_spec import Spec, Src0, Src1, sq, lower, _has_src1
from concourse.dve_uop import DveOpSpec

F32 = mybir.dt.float32
F32R = mybir.dt.float32r
AF = mybir.ActivationFunctionType

DIM = 1024
HEADS = 16
HD = 64
SCALE = HD ** -0.5
B, N = 2, 2048
HPC = 4          # heads per core
NCORES = 8
DCH = DIM // 128  # 8 contraction chunks per real/imag half
LNS = math.log(SCALE)


# ---------------------------------------------------------------- custom DVE ops
def _register_dve_op(name, spec):
    for op in dve_ops.OPS:
        if op.name == name:
            return op
    opcode = dve_ops._CUSTOM_DVE_ROW_BASE + len(dve_ops.OPS)
    shas = {}
    for ver in ("v3", "v4"):
        try:
            uops = lower(spec, ver=ver)
            s = DveOpSpec(name=name, opcode=opcode, uops=uops, rd1_en=_has_src1(spec))
            shas[ver] = s.sha(ver)
        except Exception:
            pass
    op = dve_ops.DveOp(name, spec, subdim=False, uops_sha=shas)
    dve_ops.OPS.append(op)
    dve_ops.CUSTOM_DVE_SPECS[name] = spec
    dve_ops._SUB_OPCODE_FOR_NAME[name] = opcode
    return op


SQ_OP = _register_dve_op(
    "CPLX_SQ_ANT",
    Spec(body=sq(Src0), reference=lambda in0, in1, s0, s1, imm2: in0 * in0),
)
SQADD_OP = _register_dve_op(
    "CPLX_SQADD_ANT",
    Spec(
        body=sq(Src0) + Src1,
        reference=lambda in0, in1, s0, s1, imm2: in0 * in0 + in1,
    ),
)


# ---------------------------------------------------------------- device program
def build_program(dbg=False):
    nc = bacc.Bacc("TRN2", target_bir_lowering=False, debug=False, num_devices=NCORES)

    xr = nc.dram_tensor("xr", [DIM, N], F32R, kind="ExternalInput").ap()
    xi = nc.dram_tensor("xi", [DIM, N], F32R, kind="ExternalInput").ap()
    # block-diagonal projection weights: [2*DIM, 4*128]
    wq = nc.dram_tensor("wq", [2 * DIM, 512], F32R, kind="ExternalInput").ap()
    wk = nc.dram_tensor("wk", [2 * DIM, 512], F32R, kind="ExternalInput").ap()
    wv_r = nc.dram_tensor("wv_r", [DIM, 256], F32R, kind="ExternalInput").ap()
    wv_i = nc.dram_tensor("wv_i", [DIM, 256], F32R, kind="ExternalInput").ap()
    wo_r = nc.dram_tensor("wo_r", [2, 128, DIM], F32R, kind="ExternalInput").ap()
    wo_i = nc.dram_tensor("wo_i", [2, 128, DIM], F32R, kind="ExternalInput").ap()
    bq = nc.dram_tensor("bq", [128, HPC], F32, kind="ExternalInput").ap()
    bk = nc.dram_tensor("bk", [128, HPC], F32, kind="ExternalInput").ap()
    bv = nc.dram_tensor("bv", [1, 512], F32R, kind="ExternalInput").ap()
    bo = nc.dram_tensor("bo", [128, DCH, 2], F32, kind="ExternalInput").ap()
    ones = nc.dram_tensor("ones", [128, 128], F32R, kind="ExternalInput").ap()
    out = nc.dram_tensor("out", [2, DIM, N], F32, kind="ExternalOutput").ap()
    if dbg:
        dbg_qt2 = nc.dram_tensor("dbg_qt2", [128, N], F32R, kind="ExternalOutput").ap()
        dbg_kt2 = nc.dram_tensor("dbg_kt2", [128, N], F32R, kind="ExternalOutput").ap()
        dbg_qn = nc.dram_tensor("dbg_qn", [128, N], F32R, kind="ExternalOutput").ap()
        dbg_vcat = nc.dram_tensor("dbg_vcat", [128, 16, 512], F32R,
                                  kind="ExternalOutput").ap()
        dbg_e = nc.dram_tensor("dbg_e", [128, 2, 1024], F32R,
                               kind="ExternalOutput").ap()
        dbg_rec = nc.dram_tensor("dbg_rec", [128, 2, 512], F32,
                                 kind="ExternalOutput").ap()
        dbg_o2 = nc.dram_tensor("dbg_o2", [4, 128, N], F32R,
                                kind="ExternalOutput").ap()

    with tile.TileContext(nc) as tc:
        with tc.tile_pool(name="persist", bufs=1) as pp:
            qt2 = [pp.tile([128, N], F32R, name=f"qt2_{h}") for h in range(HPC)]
            kt2 = [pp.tile([128, N], F32R, name=f"kt2_{h}") for h in range(HPC)]
            vcat = pp.tile([128, 16, 512], F32R, name="vcat")
            bq_sb = pp.tile([128, HPC], F32, name="bq_sb")
            bk_sb = pp.tile([128, HPC], F32, name="bk_sb")
            bv_sb = pp.tile([1, 512], F32R, name="bv_sb")
            bo_sb = pp.tile([128, DCH, 2], F32, name="bo_sb")
            ones_t = pp.tile([128, 128], F32R, name="ones_t")
            lns_t = pp.tile([128, 1], F32, name="lns_t")

            nc.sync.dma_start(bq_sb[:], bq)
            nc.sync.dma_start(bk_sb[:], bk)
            nc.sync.dma_start(bv_sb[:], bv)
            nc.sync.dma_start(bo_sb[:], bo)
            nc.sync.dma_start(ones_t[:], ones)
            nc.vector.memset(lns_t[:], LNS)

            # ------------------------------------------------ phase A: projections
            with (
                tc.tile_pool(name="wts", bufs=1) as wp,
                tc.tile_pool(name="xio", bufs=8) as xp,
                tc.tile_pool(name="ps_proj", bufs=1, space="PSUM") as pa,
            ):
                wq_sb = wp.tile([128, 16, 512], F32R, name="wq_sb")
                wk_sb = wp.tile([128, 16, 512], F32R, name="wk_sb")
                nc.sync.dma_start(wq_sb[:], wq.rearrange("(a p) c -> p a c", p=128))
                nc.sync.dma_start(wk_sb[:], wk.rearrange("(a p) c -> p a c", p=128))
                wvr_sb = wp.tile([128, DCH, 256], F32R, name="wvr_sb")
                wvi_sb = wp.tile([128, DCH, 256], F32R, name="wvi_sb")
                nc.sync.dma_start(wvr_sb[:], wv_r.rearrange("(a p) c -> p a c", p=128))
                nc.sync.dma_start(wvi_sb[:], wv_i.rearrange("(a p) c -> p a c", p=128))

                for ns in range(8):  # 256-wide npos slices
                    qp_ = [pa.tile([128, 2, 256], F32, tag=f"qps{i}", name=f"qps{i}_{ns}")
                           for i in range(2)]
                    kp_ = [pa.tile([128, 2, 256], F32, tag=f"kps{i}", name=f"kps{i}_{ns}")
                           for i in range(2)]
                    vpr = [pa.tile([128, 256], F32, tag=f"vpr{cc}", name=f"vpr{cc}_{ns}")
                           for cc in range(2)]
                    vpi = [pa.tile([128, 256], F32, tag=f"vpi{cc}", name=f"vpi{cc}_{ns}")
                           for cc in range(2)]
                    xtiles = []
                    for d in range(DCH):
                        xr_t = xp.tile([128, 256], F32R, tag="xr")
                        xi_t = xp.tile([128, 256], F32R, tag="xi")
                        nc.sync.dma_start(
                            xr_t[:], xr[128 * d:128 * d + 128, 256 * ns:256 * ns + 256])
                        nc.sync.dma_start(
                            xi_t[:], xi[128 * d:128 * d + 128, 256 * ns:256 * ns + 256])
                        xtiles.append((xr_t, xi_t))
                    # QT2/KT2: 16-chunk block-diag accumulation, M=128 per head.
                    # The two heads sharing a PSUM bank must form sequential
                    # (not interleaved) accumulation groups: a group's first
                    # matmul clears has_written for the whole bank.
                    for h in range(HPC):
                        hs = slice(128 * h, 128 * h + 128)
                        for dd in range(16):
                            x_t = xtiles[dd % DCH][dd // DCH]
                            st, sp = dd == 0, dd == 15
                            nc.tensor.matmul(
                                qp_[h // 2][:, h % 2, :], wq_sb[:, dd, hs], x_t[:],
                                start=st, stop=sp)
                            nc.tensor.matmul(
                                kp_[h // 2][:, h % 2, :], wk_sb[:, dd, hs], x_t[:],
                                start=st, stop=sp)
                    # V: one clean accumulation group per bank (vr, vi split)
                    for cc in range(2):
                        xs = slice(128 * cc, 128 * cc + 128)
                        for d in range(DCH):
                            nc.tensor.matmul(
                                vpr[cc][:], xtiles[d][0][:, xs], wvr_sb[:, d, :],
                                start=(d == 0), stop=False)
                        nc.tensor.matmul(
                            vpr[cc][:], ones_t[0:1, :], bv_sb[0:1, 0:256],
                            start=False, stop=True)
                        for d in range(DCH):
                            nc.tensor.matmul(
                                vpi[cc][:], xtiles[d][1][:, xs], wvi_sb[:, d, :],
                                start=(d == 0), stop=False)
                        nc.tensor.matmul(
                            vpi[cc][:], ones_t[0:1, :], bv_sb[0:1, 256:512],
                            start=False, stop=True)
                    # copies out of PSUM
                    csl = slice(256 * ns, 256 * ns + 256)
                    for h in range(HPC):
                        nc.scalar.activation(
                            qt2[h][:, csl], qp_[h // 2][:, h % 2, :], AF.Identity,
                            bias=bq_sb[:, h:h + 1])
                        nc.scalar.activation(
                            kt2[h][:, csl], kp_[h // 2][:, h % 2, :], AF.Identity,
                            bias=bk_sb[:, h:h + 1])
                    for cc in range(2):
                        cg = 2 * ns + cc
                        vr4 = vpr[cc].rearrange("p (h x) -> p h x", x=64)
                        vi4 = vpi[cc].rearrange("p (h x) -> p h x", x=64)
                        vc = vcat[:, cg, :].rearrange("p (h x) -> p h x", x=128)
                        # even heads [vr|vi]; odd heads swapped [vi|vr]
                        nc.vector.tensor_copy(vc[:, 0::2, 0:64], vr4[:, 0::2, :])
                        nc.vector.tensor_copy(vc[:, 0::2, 64:128], vi4[:, 0::2, :])
                        nc.vector.tensor_copy(vc[:, 1::2, 0:64], vi4[:, 1::2, :])
                        nc.vector.tensor_copy(vc[:, 1::2, 64:128], vr4[:, 1::2, :])

            if dbg:
                nc.sync.dma_start(dbg_qt2, qt2[0][:])
                nc.sync.dma_start(dbg_kt2, kt2[0][:])
                nc.sync.dma_start(dbg_vcat, vcat[:])

            # ------------------------------------------------ phase B: attention
            with tc.tile_pool(name="o2_pool", bufs=1) as op_:
                # o2r[p] = [or(h_even); or(h_odd)], o2i[p] = [oi(h_odd); oi(h_even)]
                o2r = [op_.tile([128, N], F32R, name=f"o2r_{p}") for p in range(2)]
                o2i = [op_.tile([128, N], F32R, name=f"o2i_{p}") for p in range(2)]
                battn = tc.tile_pool(name="m2p", bufs=2)
                m2p = battn.__enter__()
                ep_cm = tc.tile_pool(name="ep", bufs=2)
                ep = ep_cm.__enter__()
                app_cm = tc.tile_pool(name="ap_", bufs=2)
                app = app_cm.__enter__()
                rp_cm = tc.tile_pool(name="rp", bufs=2)
                rp = rp_cm.__enter__()
                pb_cm = tc.tile_pool(name="ps_attn", bufs=1, space="PSUM")
                pb = pb_cm.__enter__()
                for h in range(HPC):
                    p = h // 2
                    odd = h % 2
                    # QN = [qiT ; -qrT] from qt2 via SBUF->SBUF DMA + negate
                    qn = op_.tile([128, N], F32R, tag="qn", name=f"qn_{h}")
                    nc.sync.dma_start(qn[0:64, :], qt2[h][64:128, :])
                    nc.sync.dma_start(qn[64:128, :], qt2[h][0:64, :])
                    nc.vector.tensor_scalar_mul(qn[64:128, :], qn[64:128, :], -1.0)
                    if dbg and h == 0:
                        nc.sync.dma_start(dbg_qn, qn[:])

                    for qp in range(2):  # 1024-wide query pairs
                        psO = pb.tile([128, 2, 512], F32, tag="psO",
                                      name=f"psO_{h}_{qp}")
                        psD = pb.tile([128, 2, 512], F32, tag="psD",
                                      name=f"psD_{h}_{qp}")
                        for kk in range(8):  # kchunk pairs
                            m2b = m2p.tile([128, 2, 1024], F32, tag="m2")
                            for j in range(2):
                                k = 2 * kk + j
                                ksl = slice(128 * k, 128 * k + 128)
                                sr = pb.tile([128, 2, 512], F32, tag="sr", bufs=1,
                                             name=f"sr_{h}_{qp}_{k}")
                                si = pb.tile([128, 2, 512], F32, tag="si", bufs=1,
                                             name=f"si_{h}_{qp}_{k}")
                                for qs in range(2):
                                    qsl = slice(1024 * qp + 512 * qs,
                                                1024 * qp + 512 * qs + 512)
                                    nc.tensor.matmul(
                                        sr[:, qs, :], kt2[h][:, ksl], qt2[h][:, qsl],
                                        start=True, stop=True)
                                    nc.tensor.matmul(
                                        si[:, qs, :], kt2[h][:, ksl], qn[:, qsl],
                                        start=True, stop=True)
                                a = app.tile([128, 1024], F32, tag="a")
                                nc.vector._custom_dve(SQ_OP, out=a[:], in0=sr[:, :, :])
                                nc.vector._custom_dve(
                                    SQADD_OP, out=m2b[:, j, :], in0=si[:, :, :],
                                    in1=a[:])
                            flat = m2b[:, :, :]
                            nc.scalar.activation(flat, flat, AF.Ln)
                            nc.scalar.activation(
                                flat, flat, AF.Exp, bias=lns_t[:, 0:1], scale=0.5)
                            eb = ep.tile([128, 2, 1024], F32R, tag="e")
                            nc.scalar.activation(eb[:, :, :], flat, AF.Exp)
                            if dbg and h == 0 and qp == 0 and kk == 0:
                                nc.sync.dma_start(dbg_e, eb[:])
                            for j in range(2):
                                k = 2 * kk + j
                                st, sp = k == 0, k == 15
                                for qs in range(2):
                                    re = eb[:, j, 512 * qs:512 * qs + 512]
                                    nc.tensor.matmul(
                                        psO[:, qs, :], vcat[:, k, 128 * h:128 * h + 128],
                                        re, start=st, stop=sp)
                                    nc.tensor.matmul(
                                        psD[:, qs, :], ones_t[:], re,
                                        start=st, stop=sp)
                        rec = rp.tile([128, 2, 512], F32, tag="rec")
                        for qs in range(2):
                            nc.vector.reciprocal_approx_fast(
                                rec[:, qs, :], psD[:, qs, :])
                        if dbg and h == 0 and qp == 0:
                            nc.sync.dma_start(dbg_rec, rec[:])
                        # normalized outputs into pair-stacked o2 tiles
                        # even h: psO=[or;oi] -> o2r[0:64], o2i[64:128]
                        # odd h:  psO=[oi;or] -> o2i[0:64], o2r[64:128]
                        top, bot = (o2r[p], o2i[p]) if not odd else (o2i[p], o2r[p])
                        for qs in range(2):
                            qsl = slice(1024 * qp + 512 * qs,
                                        1024 * qp + 512 * qs + 512)
                            nc.vector.tensor_mul(
                                top[0:64, qsl] if not odd else top[0:64, qsl],
                                psO[0:64, qs, :], rec[0:64, qs, :])
                            nc.vector.tensor_mul(
                                bot[64:128, qsl],
                                psO[64:128, qs, :], rec[64:128, qs, :])

                if dbg:
                    for pi in range(2):
                        nc.sync.dma_start(dbg_o2[pi], o2r[pi][:])
                        nc.sync.dma_start(dbg_o2[2 + pi], o2i[pi][:])
                pb_cm.__exit__(None, None, None)
                rp_cm.__exit__(None, None, None)
                app_cm.__exit__(None, None, None)
                ep_cm.__exit__(None, None, None)
                battn.__exit__(None, None, None)

                # ------------------------------------------ phase C: out-proj
                with (
                    tc.tile_pool(name="wo_p", bufs=1) as wop,
                    tc.tile_pool(name="osb_p", bufs=3) as osb,
                    tc.tile_pool(name="ps_out", bufs=2, space="PSUM") as pc,
                ):
                    wo_sb = {}
                    for ri, dram in [(0, wo_r), (1, wo_i)]:
                        for p in range(2):
                            t = wop.tile([128, DIM], F32R, name=f"wo_{ri}_{p}")
                            nc.sync.dma_start(t[:], dram[p])
                            wo_sb[(ri, p)] = t
                    for ri in range(2):
                        o2 = o2r if ri == 0 else o2i
                        for dc in range(DCH):
                            for ns4 in range(4):
                                nsl = slice(512 * ns4, 512 * ns4 + 512)
                                pso = pc.tile([128, 512], F32, tag="pso",
                                              name=f"pso_{ri}_{dc}_{ns4}")
                                for p in range(2):
                                    nc.tensor.matmul(
                                        pso[:],
                                        wo_sb[(ri, p)][:, 128 * dc:128 * dc + 128],
                                        o2[p][:, nsl], start=(p == 0), stop=(p == 1))
                                ot = osb.tile([128, 512], F32, tag="osb")
                                nc.scalar.activation(
                                    ot[:], pso[:], AF.Identity,
                                    bias=bo_sb[:, dc, ri:ri + 1])
                                nc.sync.dma_start(
                                    out[ri, 128 * dc:128 * dc + 128, nsl], ot[:])

    nc.finalize()
    return nc


_NC_CACHE = None


def _get_program():
    global _NC_CACHE
    if _NC_CACHE is None:
        _NC_CACHE = build_program()
    return _NC_CACHE


# ---------------------------------------------------------------- host wrapper
def _core_inputs(inputs, c):
    b, hg = c // 4, c % 4
    h0 = HPC * hg               # first global head of this core
    r0 = 64 * h0                # first weight row within each of q/k/v blocks
    Wr = np.asarray(inputs["Wqkv_r"], np.float32)
    Wi = np.asarray(inputs["Wqkv_i"], np.float32)
    br = np.asarray(inputs["bqkv_r"], np.float32)
    bi = np.asarray(inputs["bqkv_i"], np.float32)
    Wor = np.asarray(inputs["Wout_r"], np.float32)
    Woi = np.asarray(inputs["Wout_i"], np.float32)
    bor = np.asarray(inputs["bout_r"], np.float32)
    boi = np.asarray(inputs["bout_i"], np.float32)

    def c_(a):
        return np.ascontiguousarray(a, np.float32)

    def blockdiag(Wre, Wim, row0):
        # [2*DIM, 512]: col block h holds [W_re.T head | W_im.T head] stacked
        # on the row (contraction) axis: rows 0:DIM real, DIM:2*DIM imag.
        w = np.zeros((2 * DIM, 512), np.float32)
        for j in range(HPC):
            rr = row0 + 64 * j
            w[0:DIM, 128 * j:128 * j + 64] = Wre[rr:rr + 64, :].T
            w[DIM:2 * DIM, 128 * j + 64:128 * j + 128] = Wim[rr:rr + 64, :].T
        return w

    m = {
        "xr": c_(np.asarray(inputs["x_real"], np.float32)[b].T),
        "xi": c_(np.asarray(inputs["x_imag"], np.float32)[b].T),
        "wq": blockdiag(Wr, Wi, r0),
        "wk": blockdiag(Wr, Wi, DIM + r0),
        "wv_r": c_(Wr[2 * DIM + r0:2 * DIM + r0 + 256, :].T),
        "wv_i": c_(Wi[2 * DIM + r0:2 * DIM + r0 + 256, :].T),
    }
    bq = np.zeros((128, HPC), np.float32)
    bk = np.zeros((128, HPC), np.float32)
    for j in range(HPC):
        rr = r0 + 64 * j
        bq[0:64, j] = br[rr:rr + 64]
        bq[64:128, j] = bi[rr:rr + 64]
        bk[0:64, j] = br[DIM + rr:DIM + rr + 64]
        bk[64:128, j] = bi[DIM + rr:DIM + rr + 64]
    bv = np.zeros((1, 512), np.float32)
    bv[0, 0:256] = br[2 * DIM + r0:2 * DIM + r0 + 256]
    bv[0, 256:512] = bi[2 * DIM + r0:2 * DIM + r0 + 256]
    m["bq"], m["bk"], m["bv"] = bq, bk, bv
    # out-proj pair weights: o2r = [or(h_even); or(h_odd)],
    #                        o2i = [oi(h_odd); oi(h_even)]
    wo_r = np.zeros((2, 128, DIM), np.float32)
    wo_i = np.zeros((2, 128, DIM), np.float32)
    for p in range(2):
        he = r0 + 128 * p        # col offset of h_even's hd block
        ho = he + 64
        wo_r[p, 0:64, :] = Wor[:, he:he + 64].T
        wo_r[p, 64:128, :] = Wor[:, ho:ho + 64].T
        wo_i[p, 0:64, :] = Woi[:, ho:ho + 64].T
        wo_i[p, 64:128, :] = Woi[:, he:he + 64].T
    m["wo_r"], m["wo_i"] = c_(wo_r), c_(wo_i)
    bo = np.zeros((128, DCH, 2), np.float32)
    if hg == 0:  # host sums 4 head-group cores per batch: add bias once
        bo[:, :, 0] = bor.reshape(DCH, 128).T
        bo[:, :, 1] = boi.reshape(DCH, 128).T
    m["bo"] = bo
    m["ones"] = np.ones((128, 128), np.float32)
    return m


def kernel(**inputs):
    nc = _get_program()
    in_maps = [_core_inputs(inputs, c) for c in range(NCORES)]
    res = run_bass_kernel_spmd(nc, in_maps, core_ids=list(range(NCORES)))
    outs = [r_["out"] for r_ in res.results]
    out_r = np.zeros((B, N, DIM), np.float32)
    out_i = np.zeros((B, N, DIM), np.float32)
    for c in range(NCORES):
        b = c // 4
        out_r[b] += outs[c][0].T
        out_i[b] += outs[c][1].T
    return out_r, out_i


if __name__ == "__main__":
    rng = np.random.default_rng(0)
    ins = {
        "x_real": rng.standard_normal((B, N, DIM)).astype(np.float32),
        "x_imag": rng.standard_normal((B, N, DIM)).astype(np.float32),
        "Wqkv_r": (rng.standard_normal((3 * DIM, DIM)) * DIM ** -0.5).astype(np.float32),
        "bqkv_r": (rng.standard_normal(3 * DIM) * 0.01).astype(np.float32),
        "Wqkv_i": (rng.standard_normal((3 * DIM, DIM)) * DIM ** -0.5).astype(np.float32),
        "bqkv_i": (rng.standard_normal(3 * DIM) * 0.01).astype(np.float32),
        "Wout_r": (rng.standard_normal((DIM, DIM)) * DIM ** -0.5).astype(np.float32),
        "bout_r": (rng.standard_normal(DIM) * 0.01).astype(np.float32),
        "Wout_i": (rng.standard_normal((DIM, DIM)) * DIM ** -0.5).astype(np.float32),
        "bout_i": (rng.standard_normal(DIM) * 0.01).astype(np.float32),
    }
    o_r, o_i = kernel(**ins)
    print("ran:", o_r.shape, o_i.shape, o_r[0, 0, :4], o_i[0, 0, :4])



# revision 12
# speedup vs baseline: 1.2189x; 1.2189x over previous
"""Complex multi-head attention on 8 Trainium2 NeuronCores (Bass/Tile).

Sharding: core c -> batch b = c//4, head group hg = c%4 (4 of 16 heads).
Each core computes QKV projections for its heads, per-head complex-magnitude
softmax attention, and a partial output projection; the host sums the 4
partial outputs per batch element.

Device-side layouts (per head h):
  QT2(h) = [q_r^T(h); q_i^T(h)]  [128, N]   (built by one block-diagonal
  KT2(h) = [k_r^T(h); k_i^T(h)]  [128, N]    matmul group contracting over
  QN(h)  = [q_i^T(h); -q_r^T(h)] [128, N]    [x_real^T; x_imag^T])
  S_r^T tile = KT2.T @ QT2 slices (single K=128 fp32r matmuls)
  S_i^T tile = KT2.T @ QN slices
  m2 = S_r^2 + S_i^2 (custom DVE ops), w_unnorm = exp(exp(0.5*ln(m2)+ln(S)))
  (ln/exp share one ACT table set; softmax max-subtraction is unnecessary)
  vcat(h) = [vr|vi] (even h) / [vi|vr] (odd h) -> psumO = [or;oi] / [oi;or]
  denominator via all-ones lhsT matmul (result replicated on all partitions)
"""

import math
import os
import sys

import numpy as np

sys.path.insert(0, "/opt/trn_rl_repo")

import concourse.bass as bass
import concourse.mybir as mybir
import concourse.tile as tile
import concourse.dve_ops as dve_ops
from concourse import bacc
from concourse.bass_utils import run_bass_kernel_spmd
from concourse.dve_spec import Spec, Src0, Src1, sq, lower, _has_src1
from concourse.dve_uop import DveOpSpec

F32 = mybir.dt.float32
F32R = mybir.dt.float32r
BF16 = mybir.dt.bfloat16
AF = mybir.ActivationFunctionType

DIM = 1024
HEADS = 16
HD = 64
SCALE = HD ** -0.5
B, N = 2, 2048
HPC = 4          # heads per core
NCORES = 8
DCH = DIM // 128  # 8 contraction chunks per real/imag half
LNS = math.log(SCALE)


# ---------------------------------------------------------------- custom DVE ops
def _register_dve_op(name, spec):
    for op in dve_ops.OPS:
        if op.name == name:
            return op
    opcode = dve_ops._CUSTOM_DVE_ROW_BASE + len(dve_ops.OPS)
    shas = {}
    for ver in ("v3", "v4"):
        try:
            uops = lower(spec, ver=ver)
            s = DveOpSpec(name=name, opcode=opcode, uops=uops, rd1_en=_has_src1(spec))
            shas[ver] = s.sha(ver)
        except Exception:
            pass
    op = dve_ops.DveOp(name, spec, subdim=False, uops_sha=shas)
    dve_ops.OPS.append(op)
    dve_ops.CUSTOM_DVE_SPECS[name] = spec
    dve_ops._SUB_OPCODE_FOR_NAME[name] = opcode
    return op


SQ_OP = _register_dve_op(
    "CPLX_SQ_ANT",
    Spec(body=sq(Src0), reference=lambda in0, in1, s0, s1, imm2: in0 * in0),
)
SQADD_OP = _register_dve_op(
    "CPLX_SQADD_ANT",
    Spec(
        body=sq(Src0) + Src1,
        reference=lambda in0, in1, s0, s1, imm2: in0 * in0 + in1,
    ),
)
M2_OP = _register_dve_op(
    "CPLX_M2_ANT",
    Spec(
        body=sq(Src0) + sq(Src1),
        reference=lambda in0, in1, s0, s1, imm2: in0 * in0 + in1 * in1,
    ),
)


def _patch_act_tables(nc):
    """Make Ln/Exp resolve to the one table that serves both.

    The stock table-placement pass greedily picks the first act_info table
    containing each function ('natural_log' for Ln, 'exp_and_others' for
    Exp), which costs a 1283ns ACT_TABLE_LOAD on every Ln<->Exp transition
    (129 loads = 165us in the profile). Filtering Ln/Exp out of every
    other table's advertised contents (names and list positions untouched,
    so act_func_set_id still indexes act_info.json correctly) forces both
    onto 'natural_log_exp_and_others': one load for the whole kernel.
    """
    import types

    import bass_rust as _bass_rust
    from concourse.hw_specs import get_activation_tables

    def insert_act_table_loads(self):
        has_activation = any(
            isinstance(i, mybir.InstActivation)
            for b in self.main_func.blocks
            for i in b.instructions
        )
        if not has_activation:
            return
        tables = []
        for name, funcs in get_activation_tables(self.m.arch).items():
            if name != "natural_log_exp_and_others":
                funcs = funcs - {AF.Ln, AF.Exp}
            tables.append((name, funcs))
        _bass_rust.insert_act_table_loads(self, tables)

    nc.insert_act_table_loads = types.MethodType(insert_act_table_loads, nc)


# ---------------------------------------------------------------- device program
def build_program(dbg=False):
    nc = bacc.Bacc("TRN2", target_bir_lowering=False, debug=False, num_devices=NCORES)
    _patch_act_tables(nc)

    xr = nc.dram_tensor("xr", [DIM, N], F32R, kind="ExternalInput").ap()
    xi = nc.dram_tensor("xi", [DIM, N], F32R, kind="ExternalInput").ap()
    # block-diagonal projection weights: [2*DIM, 4*128]
    wq = nc.dram_tensor("wq", [2 * DIM, 512], F32R, kind="ExternalInput").ap()
    wk = nc.dram_tensor("wk", [2 * DIM, 512], F32R, kind="ExternalInput").ap()
    wv_r = nc.dram_tensor("wv_r", [DIM, 256], F32R, kind="ExternalInput").ap()
    wv_i = nc.dram_tensor("wv_i", [DIM, 256], F32R, kind="ExternalInput").ap()
    wo_r = nc.dram_tensor("wo_r", [2, 128, DIM], F32R, kind="ExternalInput").ap()
    wo_i = nc.dram_tensor("wo_i", [2, 128, DIM], F32R, kind="ExternalInput").ap()
    bq = nc.dram_tensor("bq", [128, HPC], F32, kind="ExternalInput").ap()
    bk = nc.dram_tensor("bk", [128, HPC], F32, kind="ExternalInput").ap()
    bv = nc.dram_tensor("bv", [1, 512], F32R, kind="ExternalInput").ap()
    bo = nc.dram_tensor("bo", [128, DCH, 2], F32, kind="ExternalInput").ap()
    ones = nc.dram_tensor("ones", [128, 128], F32R, kind="ExternalInput").ap()
    out = nc.dram_tensor("out", [2, DIM, N], F32, kind="ExternalOutput").ap()
    if dbg:
        dbg_qt2 = nc.dram_tensor("dbg_qt2", [128, N], F32R, kind="ExternalOutput").ap()
        dbg_kt2 = nc.dram_tensor("dbg_kt2", [128, N], F32R, kind="ExternalOutput").ap()
        dbg_qn = nc.dram_tensor("dbg_qn", [128, N], F32R, kind="ExternalOutput").ap()
        dbg_vcat = nc.dram_tensor("dbg_vcat", [128, 16, 512], F32R,
                                  kind="ExternalOutput").ap()
        dbg_e = nc.dram_tensor("dbg_e", [128, 2, 1024], F32R,
                               kind="ExternalOutput").ap()
        dbg_rec = nc.dram_tensor("dbg_rec", [128, 2, 512], F32,
                                 kind="ExternalOutput").ap()
        dbg_o2 = nc.dram_tensor("dbg_o2", [4, 128, N], F32R,
                                kind="ExternalOutput").ap()

    with tile.TileContext(nc) as tc:
        with tc.tile_pool(name="persist", bufs=1) as pp:
            qt2 = [pp.tile([128, N], F32R, name=f"qt2_{h}") for h in range(HPC)]
            kt2 = [pp.tile([128, N], F32R, name=f"kt2_{h}") for h in range(HPC)]
            vcat = pp.tile([128, 16, 512], BF16, name="vcat")
            bq_sb = pp.tile([128, HPC], F32, name="bq_sb")
            bk_sb = pp.tile([128, HPC], F32, name="bk_sb")
            bv_sb = pp.tile([1, 512], F32R, name="bv_sb")
            bo_sb = pp.tile([128, DCH, 2], F32, name="bo_sb")
            ones_t = pp.tile([128, 128], F32R, name="ones_t")
            ones_bf = pp.tile([128, 128], BF16, name="ones_bf")
            lns_t = pp.tile([128, 1], F32, name="lns_t")

            nc.sync.dma_start(bq_sb[:], bq)
            nc.sync.dma_start(bk_sb[:], bk)
            nc.sync.dma_start(bv_sb[:], bv)
            nc.sync.dma_start(bo_sb[:], bo)
            nc.sync.dma_start(ones_t[:], ones)
            nc.vector.tensor_copy(ones_bf[:], ones_t[:])
            nc.vector.memset(lns_t[:], LNS)

            # ------------------------------------------------ phase A: projections
            with (
                tc.tile_pool(name="wts", bufs=1) as wp,
                tc.tile_pool(name="xio", bufs=8) as xp,
                tc.tile_pool(name="ps_proj", bufs=1, space="PSUM") as pa,
            ):
                wq_sb = wp.tile([128, 16, 512], F32R, name="wq_sb")
                wk_sb = wp.tile([128, 16, 512], F32R, name="wk_sb")
                nc.sync.dma_start(wq_sb[:], wq.rearrange("(a p) c -> p a c", p=128))
                nc.sync.dma_start(wk_sb[:], wk.rearrange("(a p) c -> p a c", p=128))
                wvr_sb = wp.tile([128, DCH, 256], F32R, name="wvr_sb")
                wvi_sb = wp.tile([128, DCH, 256], F32R, name="wvi_sb")
                nc.sync.dma_start(wvr_sb[:], wv_r.rearrange("(a p) c -> p a c", p=128))
                nc.sync.dma_start(wvi_sb[:], wv_i.rearrange("(a p) c -> p a c", p=128))

                for ns in range(8):  # 256-wide npos slices
                    qp_ = [pa.tile([128, 2, 256], F32, tag=f"qps{i}", name=f"qps{i}_{ns}")
                           for i in range(2)]
                    kp_ = [pa.tile([128, 2, 256], F32, tag=f"kps{i}", name=f"kps{i}_{ns}")
                           for i in range(2)]
                    vpr = [pa.tile([128, 256], F32, tag=f"vpr{cc}", name=f"vpr{cc}_{ns}")
                           for cc in range(2)]
                    vpi = [pa.tile([128, 256], F32, tag=f"vpi{cc}", name=f"vpi{cc}_{ns}")
                           for cc in range(2)]
                    xtiles = []
                    for d in range(DCH):
                        xr_t = xp.tile([128, 256], F32R, tag="xr")
                        xi_t = xp.tile([128, 256], F32R, tag="xi")
                        nc.sync.dma_start(
                            xr_t[:], xr[128 * d:128 * d + 128, 256 * ns:256 * ns + 256])
                        nc.sync.dma_start(
                            xi_t[:], xi[128 * d:128 * d + 128, 256 * ns:256 * ns + 256])
                        xtiles.append((xr_t, xi_t))
                    # QT2/KT2: 16-chunk block-diag accumulation, M=128 per head.
                    # The two heads sharing a PSUM bank must form sequential
                    # (not interleaved) accumulation groups: a group's first
                    # matmul clears has_written for the whole bank.
                    for h in range(HPC):
                        hs = slice(128 * h, 128 * h + 128)
                        for dd in range(16):
                            x_t = xtiles[dd % DCH][dd // DCH]
                            st, sp = dd == 0, dd == 15
                            nc.tensor.matmul(
                                qp_[h // 2][:, h % 2, :], wq_sb[:, dd, hs], x_t[:],
                                start=st, stop=sp)
                            nc.tensor.matmul(
                                kp_[h // 2][:, h % 2, :], wk_sb[:, dd, hs], x_t[:],
                                start=st, stop=sp)
                    # V: one clean accumulation group per bank (vr, vi split)
                    for cc in range(2):
                        xs = slice(128 * cc, 128 * cc + 128)
                        for d in range(DCH):
                            nc.tensor.matmul(
                                vpr[cc][:], xtiles[d][0][:, xs], wvr_sb[:, d, :],
                                start=(d == 0), stop=False)
                        nc.tensor.matmul(
                            vpr[cc][:], ones_t[0:1, :], bv_sb[0:1, 0:256],
                            start=False, stop=True)
                        for d in range(DCH):
                            nc.tensor.matmul(
                                vpi[cc][:], xtiles[d][1][:, xs], wvi_sb[:, d, :],
                                start=(d == 0), stop=False)
                        nc.tensor.matmul(
                            vpi[cc][:], ones_t[0:1, :], bv_sb[0:1, 256:512],
                            start=False, stop=True)
                    # copies out of PSUM
                    csl = slice(256 * ns, 256 * ns + 256)
                    for h in range(HPC):
                        nc.scalar.activation(
                            qt2[h][:, csl], qp_[h // 2][:, h % 2, :], AF.Identity,
                            bias=bq_sb[:, h:h + 1])
                        nc.scalar.activation(
                            kt2[h][:, csl], kp_[h // 2][:, h % 2, :], AF.Identity,
                            bias=bk_sb[:, h:h + 1])
                    for cc in range(2):
                        cg = 2 * ns + cc
                        vr4 = vpr[cc].rearrange("p (h x) -> p h x", x=64)
                        vi4 = vpi[cc].rearrange("p (h x) -> p h x", x=64)
                        vc = vcat[:, cg, :].rearrange("p (h x) -> p h x", x=128)
                        # even heads [vr|vi]; odd heads swapped [vi|vr]
                        nc.vector.tensor_copy(vc[:, 0::2, 0:64], vr4[:, 0::2, :])
                        nc.vector.tensor_copy(vc[:, 0::2, 64:128], vi4[:, 0::2, :])
                        nc.vector.tensor_copy(vc[:, 1::2, 0:64], vi4[:, 1::2, :])
                        nc.vector.tensor_copy(vc[:, 1::2, 64:128], vr4[:, 1::2, :])

            if dbg:
                nc.sync.dma_start(dbg_qt2, qt2[0][:])
                nc.sync.dma_start(dbg_kt2, kt2[0][:])
                nc.sync.dma_start(dbg_vcat, vcat[:])

            # ------------------------------------------------ phase B: attention
            with tc.tile_pool(name="o2_pool", bufs=1) as op_:
                # o2r[p] = [or(h_even); or(h_odd)], o2i[p] = [oi(h_odd); oi(h_even)]
                o2r = [op_.tile([128, N], F32R, name=f"o2r_{p}") for p in range(2)]
                o2i = [op_.tile([128, N], F32R, name=f"o2i_{p}") for p in range(2)]
                battn = tc.tile_pool(name="m2p", bufs=2)
                m2p = battn.__enter__()
                ep_cm = tc.tile_pool(name="ep", bufs=4)
                ep = ep_cm.__enter__()
                app_cm = tc.tile_pool(name="ap_", bufs=2)
                app = app_cm.__enter__()
                rp_cm = tc.tile_pool(name="rp", bufs=2)
                rp = rp_cm.__enter__()
                pb_cm = tc.tile_pool(name="ps_attn", bufs=1, space="PSUM")
                pb = pb_cm.__enter__()
                for h in range(HPC):
                    p = h // 2
                    odd = h % 2
                    # QN = [qiT ; -qrT] from qt2 via SBUF->SBUF DMA + negate
                    qn = op_.tile([128, N], F32R, tag="qn", name=f"qn_{h}")
                    nc.sync.dma_start(qn[0:64, :], qt2[h][64:128, :])
                    nc.sync.dma_start(qn[64:128, :], qt2[h][0:64, :])
                    nc.vector.tensor_scalar_mul(qn[64:128, :], qn[64:128, :], -1.0)
                    if dbg and h == 0:
                        nc.sync.dma_start(dbg_qn, qn[:])

                    for qp in range(2):  # 1024-wide query pairs
                        psO = pb.tile([128, 2, 512], F32, tag="psO",
                                      name=f"psO_{h}_{qp}")
                        psD = pb.tile([128, 2, 512], F32, tag="psD",
                                      name=f"psD_{h}_{qp}")

                        # psO/psD consumption of e lags score production by
                        # LAG kchunk-pairs, so the PE has queued matmul work
                        # to run while DVE m2 + 3xACT produce e for earlier
                        # chunks (the unpipelined loop stalled the PE ~4us
                        # per chunk waiting on that chain).
                        def emit_psOD(kk_e, eb_e):
                            for j in range(2):
                                k = 2 * kk_e + j
                                st, sp = k == 0, k == 15
                                for qs in range(2):
                                    re = eb_e[:, j, 512 * qs:512 * qs + 512]
                                    nc.tensor.matmul(
                                        psO[:, qs, :],
                                        vcat[:, k, 128 * h:128 * h + 128],
                                        re, start=st, stop=sp)
                                    nc.tensor.matmul(
                                        psD[:, qs, :], ones_bf[:], re,
                                        start=st, stop=sp)

                        LAG = 2
                        pend = []
                        for kk in range(8):  # kchunk pairs
                            m2b = m2p.tile([128, 2, 1024], F32, tag="m2")
                            for j in range(2):
                                k = 2 * kk + j
                                ksl = slice(128 * k, 128 * k + 128)
                                sr = pb.tile([128, 2, 512], F32, tag="sr", bufs=1,
                                             name=f"sr_{h}_{qp}_{k}")
                                si = pb.tile([128, 2, 512], F32, tag="si", bufs=1,
                                             name=f"si_{h}_{qp}_{k}")
                                for qs in range(2):
                                    qsl = slice(1024 * qp + 512 * qs,
                                                1024 * qp + 512 * qs + 512)
                                    nc.tensor.matmul(
                                        sr[:, qs, :], kt2[h][:, ksl], qt2[h][:, qsl],
                                        start=True, stop=True)
                                    nc.tensor.matmul(
                                        si[:, qs, :], kt2[h][:, ksl], qn[:, qsl],
                                        start=True, stop=True)
                                # DVE reads at most one non-scalar PSUM input,
                                # so m2 = sr^2 + si^2 takes two ops via SBUF.
                                a = app.tile([128, 1024], F32, tag="a")
                                nc.vector._custom_dve(SQ_OP, out=a[:], in0=sr[:, :, :])
                                nc.vector._custom_dve(
                                    SQADD_OP, out=m2b[:, j, :], in0=si[:, :, :],
                                    in1=a[:])
                            flat = m2b[:, :, :]
                            nc.scalar.activation(flat, flat, AF.Ln)
                            nc.scalar.activation(
                                flat, flat, AF.Exp, bias=lns_t[:, 0:1], scale=0.5)
                            eb = ep.tile([128, 2, 1024], BF16, tag="e")
                            nc.scalar.activation(eb[:, :, :], flat, AF.Exp)
                            pend.append((kk, eb))
                            if len(pend) > LAG:
                                emit_psOD(*pend.pop(0))
                        for item in pend:
                            emit_psOD(*item)
                        rec = rp.tile([128, 2, 512], F32, tag="rec")
                        for qs in range(2):
                            nc.vector.reciprocal_approx_fast(
                                rec[:, qs, :], psD[:, qs, :])
                        if dbg and h == 0 and qp == 0:
                            nc.sync.dma_start(dbg_rec, rec[:])
                        # normalized outputs into pair-stacked o2 tiles
                        # even h: psO=[or;oi] -> o2r[0:64], o2i[64:128]
                        # odd h:  psO=[oi;or] -> o2i[0:64], o2r[64:128]
                        top, bot = (o2r[p], o2i[p]) if not odd else (o2i[p], o2r[p])
                        for qs in range(2):
                            qsl = slice(1024 * qp + 512 * qs,
                                        1024 * qp + 512 * qs + 512)
                            nc.vector.tensor_mul(
                                top[0:64, qsl] if not odd else top[0:64, qsl],
                                psO[0:64, qs, :], rec[0:64, qs, :])
                            nc.vector.tensor_mul(
                                bot[64:128, qsl],
                                psO[64:128, qs, :], rec[64:128, qs, :])

                if dbg:
                    for pi in range(2):
                        nc.sync.dma_start(dbg_o2[pi], o2r[pi][:])
                        nc.sync.dma_start(dbg_o2[2 + pi], o2i[pi][:])
                pb_cm.__exit__(None, None, None)
                rp_cm.__exit__(None, None, None)
                app_cm.__exit__(None, None, None)
                ep_cm.__exit__(None, None, None)
                battn.__exit__(None, None, None)

                # ------------------------------------------ phase C: out-proj
                with (
                    tc.tile_pool(name="wo_p", bufs=1) as wop,
                    tc.tile_pool(name="osb_p", bufs=3) as osb,
                    tc.tile_pool(name="ps_out", bufs=2, space="PSUM") as pc,
                ):
                    wo_sb = {}
                    for ri, dram in [(0, wo_r), (1, wo_i)]:
                        for p in range(2):
                            t = wop.tile([128, DIM], F32R, name=f"wo_{ri}_{p}")
                            nc.sync.dma_start(t[:], dram[p])
                            wo_sb[(ri, p)] = t
                    for ri in range(2):
                        o2 = o2r if ri == 0 else o2i
                        for dc in range(DCH):
                            for ns4 in range(4):
                                nsl = slice(512 * ns4, 512 * ns4 + 512)
                                pso = pc.tile([128, 512], F32, tag="pso",
                                              name=f"pso_{ri}_{dc}_{ns4}")
                                for p in range(2):
                                    nc.tensor.matmul(
                                        pso[:],
                                        wo_sb[(ri, p)][:, 128 * dc:128 * dc + 128],
                                        o2[p][:, nsl], start=(p == 0), stop=(p == 1))
                                ot = osb.tile([128, 512], F32, tag="osb")
                                nc.scalar.activation(
                                    ot[:], pso[:], AF.Identity,
                                    bias=bo_sb[:, dc, ri:ri + 1])
                                nc.sync.dma_start(
                                    out[ri, 128 * dc:128 * dc + 128, nsl], ot[:])

    nc.finalize()
    return nc


_NC_CACHE = None


def _get_program():
    global _NC_CACHE
    if _NC_CACHE is None:
        _NC_CACHE = build_program()
    return _NC_CACHE


# ---------------------------------------------------------------- host wrapper
def _core_inputs(inputs, c):
    b, hg = c // 4, c % 4
    h0 = HPC * hg               # first global head of this core
    r0 = 64 * h0                # first weight row within each of q/k/v blocks
    Wr = np.asarray(inputs["Wqkv_r"], np.float32)
    Wi = np.asarray(inputs["Wqkv_i"], np.float32)
    br = np.asarray(inputs["bqkv_r"], np.float32)
    bi = np.asarray(inputs["bqkv_i"], np.float32)
    Wor = np.asarray(inputs["Wout_r"], np.float32)
    Woi = np.asarray(inputs["Wout_i"], np.float32)
    bor = np.asarray(inputs["bout_r"], np.float32)
    boi = np.asarray(inputs["bout_i"], np.float32)

    def c_(a):
        return np.ascontiguousarray(a, np.float32)

    def blockdiag(Wre, Wim, row0):
        # [2*DIM, 512]: col block h holds [W_re.T head | W_im.T head] stacked
        # on the row (contraction) axis: rows 0:DIM real, DIM:2*DIM imag.
        w = np.zeros((2 * DIM, 512), np.float32)
        for j in range(HPC):
            rr = row0 + 64 * j
            w[0:DIM, 128 * j:128 * j + 64] = Wre[rr:rr + 64, :].T
            w[DIM:2 * DIM, 128 * j + 64:128 * j + 128] = Wim[rr:rr + 64, :].T
        return w

    m = {
        "xr": c_(np.asarray(inputs["x_real"], np.float32)[b].T),
        "xi": c_(np.asarray(inputs["x_imag"], np.float32)[b].T),
        "wq": blockdiag(Wr, Wi, r0),
        "wk": blockdiag(Wr, Wi, DIM + r0),
        "wv_r": c_(Wr[2 * DIM + r0:2 * DIM + r0 + 256, :].T),
        "wv_i": c_(Wi[2 * DIM + r0:2 * DIM + r0 + 256, :].T),
    }
    bq = np.zeros((128, HPC), np.float32)
    bk = np.zeros((128, HPC), np.float32)
    for j in range(HPC):
        rr = r0 + 64 * j
        bq[0:64, j] = br[rr:rr + 64]
        bq[64:128, j] = bi[rr:rr + 64]
        bk[0:64, j] = br[DIM + rr:DIM + rr + 64]
        bk[64:128, j] = bi[DIM + rr:DIM + rr + 64]
    bv = np.zeros((1, 512), np.float32)
    bv[0, 0:256] = br[2 * DIM + r0:2 * DIM + r0 + 256]
    bv[0, 256:512] = bi[2 * DIM + r0:2 * DIM + r0 + 256]
    m["bq"], m["bk"], m["bv"] = bq, bk, bv
    # out-proj pair weights: o2r = [or(h_even); or(h_odd)],
    #                        o2i = [oi(h_odd); oi(h_even)]
    wo_r = np.zeros((2, 128, DIM), np.float32)
    wo_i = np.zeros((2, 128, DIM), np.float32)
    for p in range(2):
        he = r0 + 128 * p        # col offset of h_even's hd block
        ho = he + 64
        wo_r[p, 0:64, :] = Wor[:, he:he + 64].T
        wo_r[p, 64:128, :] = Wor[:, ho:ho + 64].T
        wo_i[p, 0:64, :] = Woi[:, ho:ho + 64].T
        wo_i[p, 64:128, :] = Woi[:, he:he + 64].T
    m["wo_r"], m["wo_i"] = c_(wo_r), c_(wo_i)
    bo = np.zeros((128, DCH, 2), np.float32)
    if hg == 0:  # host sums 4 head-group cores per batch: add bias once
        bo[:, :, 0] = bor.reshape(DCH, 128).T
        bo[:, :, 1] = boi.reshape(DCH, 128).T
    m["bo"] = bo
    m["ones"] = np.ones((128, 128), np.float32)
    return m


def kernel(**inputs):
    nc = _get_program()
    in_maps = [_core_inputs(inputs, c) for c in range(NCORES)]
    res = run_bass_kernel_spmd(nc, in_maps, core_ids=list(range(NCORES)))
    outs = [r_["out"] for r_ in res.results]
    out_r = np.zeros((B, N, DIM), np.float32)
    out_i = np.zeros((B, N, DIM), np.float32)
    for c in range(NCORES):
        b = c // 4
        out_r[b] += outs[c][0].T
        out_i[b] += outs[c][1].T
    return out_r, out_i


if __name__ == "__main__":
    rng = np.random.default_rng(0)
    ins = {
        "x_real": rng.standard_normal((B, N, DIM)).astype(np.float32),
        "x_imag": rng.standard_normal((B, N, DIM)).astype(np.float32),
        "Wqkv_r": (rng.standard_normal((3 * DIM, DIM)) * DIM ** -0.5).astype(np.float32),
        "bqkv_r": (rng.standard_normal(3 * DIM) * 0.01).astype(np.float32),
        "Wqkv_i": (rng.standard_normal((3 * DIM, DIM)) * DIM ** -0.5).astype(np.float32),
        "bqkv_i": (rng.standard_normal(3 * DIM) * 0.01).astype(np.float32),
        "Wout_r": (rng.standard_normal((DIM, DIM)) * DIM ** -0.5).astype(np.float32),
        "bout_r": (rng.standard_normal(DIM) * 0.01).astype(np.float32),
        "Wout_i": (rng.standard_normal((DIM, DIM)) * DIM ** -0.5).astype(np.float32),
        "bout_i": (rng.standard_normal(DIM) * 0.01).astype(np.float32),
    }
    o_r, o_i = kernel(**ins)
    print("ran:", o_r.shape, o_i.shape, o_r[0, 0, :4], o_i[0, 0, :4])



# revision 14
# speedup vs baseline: 1.2585x; 1.0325x over previous
"""Complex multi-head attention on 8 Trainium2 NeuronCores (Bass/Tile).

Sharding: core c -> batch b = c//4, head group hg = c%4 (4 of 16 heads).
Each core computes QKV projections for its heads, per-head complex-magnitude
softmax attention, and a partial output projection; the host sums the 4
partial outputs per batch element.

Device-side layouts (per head h):
  QT2(h) = [q_r^T(h); q_i^T(h)]  [128, N]   (built by one block-diagonal
  KT2(h) = [k_r^T(h); k_i^T(h)]  [128, N]    matmul group contracting over
  QN(h)  = [q_i^T(h); -q_r^T(h)] [128, N]    [x_real^T; x_imag^T])
  S_r^T tile = KT2.T @ QT2 slices (single K=128 fp32r matmuls)
  S_i^T tile = KT2.T @ QN slices
  m2 = S_r^2 + S_i^2 (custom DVE ops), w_unnorm = exp(exp(0.5*ln(m2)+ln(S)))
  (ln/exp share one ACT table set; softmax max-subtraction is unnecessary)
  vcat(h) = [vr|vi] (even h) / [vi|vr] (odd h) -> psumO = [or;oi] / [oi;or]
  denominator via all-ones lhsT matmul (result replicated on all partitions)
"""

import math
import os
import sys

import numpy as np

sys.path.insert(0, "/opt/trn_rl_repo")

import concourse.bass as bass
import concourse.mybir as mybir
import concourse.tile as tile
import concourse.dve_ops as dve_ops
from concourse import bacc
from concourse.bass_utils import run_bass_kernel_spmd
from concourse.dve_spec import Spec, Src0, Src1, sq, lower, _has_src1
from concourse.dve_uop import DveOpSpec

F32 = mybir.dt.float32
F32R = mybir.dt.float32r
BF16 = mybir.dt.bfloat16
AF = mybir.ActivationFunctionType

DIM = 1024
HEADS = 16
HD = 64
SCALE = HD ** -0.5
B, N = 2, 2048
HPC = 4          # heads per core
NCORES = 8
DCH = DIM // 128  # 8 contraction chunks per real/imag half
LNS = math.log(SCALE)


# ---------------------------------------------------------------- custom DVE ops
def _register_dve_op(name, spec):
    for op in dve_ops.OPS:
        if op.name == name:
            return op
    opcode = dve_ops._CUSTOM_DVE_ROW_BASE + len(dve_ops.OPS)
    shas = {}
    for ver in ("v3", "v4"):
        try:
            uops = lower(spec, ver=ver)
            s = DveOpSpec(name=name, opcode=opcode, uops=uops, rd1_en=_has_src1(spec))
            shas[ver] = s.sha(ver)
        except Exception:
            pass
    op = dve_ops.DveOp(name, spec, subdim=False, uops_sha=shas)
    dve_ops.OPS.append(op)
    dve_ops.CUSTOM_DVE_SPECS[name] = spec
    dve_ops._SUB_OPCODE_FOR_NAME[name] = opcode
    return op


SQ_OP = _register_dve_op(
    "CPLX_SQ_ANT",
    Spec(body=sq(Src0), reference=lambda in0, in1, s0, s1, imm2: in0 * in0),
)
SQADD_OP = _register_dve_op(
    "CPLX_SQADD_ANT",
    Spec(
        body=sq(Src0) + Src1,
        reference=lambda in0, in1, s0, s1, imm2: in0 * in0 + in1,
    ),
)
M2_OP = _register_dve_op(
    "CPLX_M2_ANT",
    Spec(
        body=sq(Src0) + sq(Src1),
        reference=lambda in0, in1, s0, s1, imm2: in0 * in0 + in1 * in1,
    ),
)


def _patch_act_tables(nc):
    """Make Ln/Exp resolve to the one table that serves both.

    The stock table-placement pass greedily picks the first act_info table
    containing each function ('natural_log' for Ln, 'exp_and_others' for
    Exp), which costs a 1283ns ACT_TABLE_LOAD on every Ln<->Exp transition
    (129 loads = 165us in the profile). Filtering Ln/Exp out of every
    other table's advertised contents (names and list positions untouched,
    so act_func_set_id still indexes act_info.json correctly) forces both
    onto 'natural_log_exp_and_others': one load for the whole kernel.
    """
    import types

    import bass_rust as _bass_rust
    from concourse.hw_specs import get_activation_tables

    def insert_act_table_loads(self):
        has_activation = any(
            isinstance(i, mybir.InstActivation)
            for b in self.main_func.blocks
            for i in b.instructions
        )
        if not has_activation:
            return
        tables = []
        for name, funcs in get_activation_tables(self.m.arch).items():
            if name != "natural_log_exp_and_others":
                funcs = funcs - {AF.Ln, AF.Exp}
            tables.append((name, funcs))
        _bass_rust.insert_act_table_loads(self, tables)

    nc.insert_act_table_loads = types.MethodType(insert_act_table_loads, nc)


# ---------------------------------------------------------------- device program
def build_program(dbg=False):
    nc = bacc.Bacc("TRN2", target_bir_lowering=False, debug=False, num_devices=NCORES)
    _patch_act_tables(nc)

    xr = nc.dram_tensor("xr", [DIM, N], F32R, kind="ExternalInput").ap()
    xi = nc.dram_tensor("xi", [DIM, N], F32R, kind="ExternalInput").ap()
    # block-diagonal projection weights: [2*DIM, 4*128]
    wq = nc.dram_tensor("wq", [2 * DIM, 512], F32R, kind="ExternalInput").ap()
    wk = nc.dram_tensor("wk", [2 * DIM, 512], F32R, kind="ExternalInput").ap()
    wv_r = nc.dram_tensor("wv_r", [DIM, 256], F32R, kind="ExternalInput").ap()
    wv_i = nc.dram_tensor("wv_i", [DIM, 256], F32R, kind="ExternalInput").ap()
    wo_r = nc.dram_tensor("wo_r", [2, 128, DIM], F32R, kind="ExternalInput").ap()
    wo_i = nc.dram_tensor("wo_i", [2, 128, DIM], F32R, kind="ExternalInput").ap()
    bq = nc.dram_tensor("bq", [128, HPC], F32, kind="ExternalInput").ap()
    bk = nc.dram_tensor("bk", [128, HPC], F32, kind="ExternalInput").ap()
    bv = nc.dram_tensor("bv", [1, 512], F32R, kind="ExternalInput").ap()
    bo = nc.dram_tensor("bo", [128, DCH, 2], F32, kind="ExternalInput").ap()
    ones = nc.dram_tensor("ones", [128, 128], F32R, kind="ExternalInput").ap()
    out = nc.dram_tensor("out", [2, DIM, N], F32, kind="ExternalOutput").ap()
    if dbg:
        dbg_qt2 = nc.dram_tensor("dbg_qt2", [128, N], F32R, kind="ExternalOutput").ap()
        dbg_kt2 = nc.dram_tensor("dbg_kt2", [128, N], F32R, kind="ExternalOutput").ap()
        dbg_qn = nc.dram_tensor("dbg_qn", [128, N], F32R, kind="ExternalOutput").ap()
        dbg_vcat = nc.dram_tensor("dbg_vcat", [128, 16, 512], F32R,
                                  kind="ExternalOutput").ap()
        dbg_e = nc.dram_tensor("dbg_e", [128, 2, 1024], F32R,
                               kind="ExternalOutput").ap()
        dbg_rec = nc.dram_tensor("dbg_rec", [128, 2, 512], F32,
                                 kind="ExternalOutput").ap()
        dbg_o2 = nc.dram_tensor("dbg_o2", [4, 128, N], F32R,
                                kind="ExternalOutput").ap()

    with tile.TileContext(nc) as tc:
        with tc.tile_pool(name="persist", bufs=1) as pp:
            qt2 = [pp.tile([128, N], F32R, name=f"qt2_{h}") for h in range(HPC)]
            kt2 = [pp.tile([128, N], F32R, name=f"kt2_{h}") for h in range(HPC)]
            vcat = pp.tile([128, 16, 512], BF16, name="vcat")
            bq_sb = pp.tile([128, HPC], F32, name="bq_sb")
            bk_sb = pp.tile([128, HPC], F32, name="bk_sb")
            bv_sb = pp.tile([1, 512], F32R, name="bv_sb")
            bo_sb = pp.tile([128, DCH, 2], F32, name="bo_sb")
            ones_t = pp.tile([128, 128], F32R, name="ones_t")
            ones_bf = pp.tile([128, 128], BF16, name="ones_bf")
            lns_t = pp.tile([128, 1], F32, name="lns_t")

            nc.sync.dma_start(bq_sb[:], bq)
            nc.sync.dma_start(bk_sb[:], bk)
            nc.sync.dma_start(bv_sb[:], bv)
            nc.sync.dma_start(bo_sb[:], bo)
            nc.sync.dma_start(ones_t[:], ones)
            nc.vector.tensor_copy(ones_bf[:], ones_t[:])
            nc.vector.memset(lns_t[:], LNS)

            # ------------------------------------------------ phase A: projections
            with (
                tc.tile_pool(name="wts", bufs=1) as wp,
                tc.tile_pool(name="xio", bufs=8) as xp,
                tc.tile_pool(name="ps_proj", bufs=1, space="PSUM") as pa,
            ):
                wq_sb = wp.tile([128, 16, 512], F32R, name="wq_sb")
                wk_sb = wp.tile([128, 16, 512], F32R, name="wk_sb")
                nc.sync.dma_start(wq_sb[:], wq.rearrange("(a p) c -> p a c", p=128))
                nc.sync.dma_start(wk_sb[:], wk.rearrange("(a p) c -> p a c", p=128))
                wvr_sb = wp.tile([128, DCH, 256], F32R, name="wvr_sb")
                wvi_sb = wp.tile([128, DCH, 256], F32R, name="wvi_sb")
                nc.sync.dma_start(wvr_sb[:], wv_r.rearrange("(a p) c -> p a c", p=128))
                nc.sync.dma_start(wvi_sb[:], wv_i.rearrange("(a p) c -> p a c", p=128))

                for ns in range(8):  # 256-wide npos slices
                    qp_ = [pa.tile([128, 2, 256], F32, tag=f"qps{i}", name=f"qps{i}_{ns}")
                           for i in range(2)]
                    kp_ = [pa.tile([128, 2, 256], F32, tag=f"kps{i}", name=f"kps{i}_{ns}")
                           for i in range(2)]
                    vpr = [pa.tile([128, 256], F32, tag=f"vpr{cc}", name=f"vpr{cc}_{ns}")
                           for cc in range(2)]
                    vpi = [pa.tile([128, 256], F32, tag=f"vpi{cc}", name=f"vpi{cc}_{ns}")
                           for cc in range(2)]
                    xtiles = []
                    for d in range(DCH):
                        xr_t = xp.tile([128, 256], F32R, tag="xr")
                        xi_t = xp.tile([128, 256], F32R, tag="xi")
                        nc.sync.dma_start(
                            xr_t[:], xr[128 * d:128 * d + 128, 256 * ns:256 * ns + 256])
                        nc.sync.dma_start(
                            xi_t[:], xi[128 * d:128 * d + 128, 256 * ns:256 * ns + 256])
                        xtiles.append((xr_t, xi_t))
                    # QT2/KT2: 16-chunk block-diag accumulation, M=128 per head.
                    # The two heads sharing a PSUM bank must form sequential
                    # (not interleaved) accumulation groups: a group's first
                    # matmul clears has_written for the whole bank.
                    for h in range(HPC):
                        hs = slice(128 * h, 128 * h + 128)
                        for dd in range(16):
                            x_t = xtiles[dd % DCH][dd // DCH]
                            st, sp = dd == 0, dd == 15
                            nc.tensor.matmul(
                                qp_[h // 2][:, h % 2, :], wq_sb[:, dd, hs], x_t[:],
                                start=st, stop=sp)
                            nc.tensor.matmul(
                                kp_[h // 2][:, h % 2, :], wk_sb[:, dd, hs], x_t[:],
                                start=st, stop=sp)
                    # V: one clean accumulation group per bank (vr, vi split)
                    for cc in range(2):
                        xs = slice(128 * cc, 128 * cc + 128)
                        for d in range(DCH):
                            nc.tensor.matmul(
                                vpr[cc][:], xtiles[d][0][:, xs], wvr_sb[:, d, :],
                                start=(d == 0), stop=False)
                        nc.tensor.matmul(
                            vpr[cc][:], ones_t[0:1, :], bv_sb[0:1, 0:256],
                            start=False, stop=True)
                        for d in range(DCH):
                            nc.tensor.matmul(
                                vpi[cc][:], xtiles[d][1][:, xs], wvi_sb[:, d, :],
                                start=(d == 0), stop=False)
                        nc.tensor.matmul(
                            vpi[cc][:], ones_t[0:1, :], bv_sb[0:1, 256:512],
                            start=False, stop=True)
                    # copies out of PSUM
                    csl = slice(256 * ns, 256 * ns + 256)
                    for h in range(HPC):
                        nc.scalar.activation(
                            qt2[h][:, csl], qp_[h // 2][:, h % 2, :], AF.Identity,
                            bias=bq_sb[:, h:h + 1])
                        nc.scalar.activation(
                            kt2[h][:, csl], kp_[h // 2][:, h % 2, :], AF.Identity,
                            bias=bk_sb[:, h:h + 1])
                    for cc in range(2):
                        cg = 2 * ns + cc
                        vr4 = vpr[cc].rearrange("p (h x) -> p h x", x=64)
                        vi4 = vpi[cc].rearrange("p (h x) -> p h x", x=64)
                        vc = vcat[:, cg, :].rearrange("p (h x) -> p h x", x=128)
                        # even heads [vr|vi]; odd heads swapped [vi|vr]
                        nc.vector.tensor_copy(vc[:, 0::2, 0:64], vr4[:, 0::2, :])
                        nc.vector.tensor_copy(vc[:, 0::2, 64:128], vi4[:, 0::2, :])
                        nc.vector.tensor_copy(vc[:, 1::2, 0:64], vi4[:, 1::2, :])
                        nc.vector.tensor_copy(vc[:, 1::2, 64:128], vr4[:, 1::2, :])

            if dbg:
                nc.sync.dma_start(dbg_qt2, qt2[0][:])
                nc.sync.dma_start(dbg_kt2, kt2[0][:])
                nc.sync.dma_start(dbg_vcat, vcat[:])

            # ------------------------------------------------ phase B: attention
            with tc.tile_pool(name="o2_pool", bufs=1) as op_:
                # o2r[p] = [or(h_even); or(h_odd)], o2i[p] = [oi(h_odd); oi(h_even)]
                o2r = [op_.tile([128, N], F32R, name=f"o2r_{p}") for p in range(2)]
                o2i = [op_.tile([128, N], F32R, name=f"o2i_{p}") for p in range(2)]
                battn = tc.tile_pool(name="m2p", bufs=2)
                m2p = battn.__enter__()
                ep_cm = tc.tile_pool(name="ep", bufs=5)
                ep = ep_cm.__enter__()
                app_cm = tc.tile_pool(name="ap_", bufs=2)
                app = app_cm.__enter__()
                rp_cm = tc.tile_pool(name="rp", bufs=2)
                rp = rp_cm.__enter__()
                pb_cm = tc.tile_pool(name="ps_attn", bufs=1, space="PSUM")
                pb = pb_cm.__enter__()
                # psO/psD emission runs through a single FIFO that lags score
                # production by LAGJ 128-key chunks and persists across
                # qp/head boundaries: while the softmax chain (DVE m2 +
                # 3xACT) produces e for the tail chunks of one query block,
                # the PE is already running the next block's score matmuls,
                # and the old block's psO/psD accumulation (plus its
                # normalize, attached to the stop-matmul item) drains
                # in between. The unpipelined loop stalled the PE ~4us per
                # chunk; the per-qp flush stalled it ~4us per block.
                pend = []

                def emit_one():
                    psO_, psD_, vc_ap, eb_e, j_e, st, sp, post = pend.pop(0)
                    for qs in range(2):
                        re = eb_e[:, j_e, 512 * qs:512 * qs + 512]
                        nc.tensor.matmul(psO_[:, qs, :], vc_ap, re,
                                         start=st, stop=sp)
                        nc.tensor.matmul(psD_[:, qs, :], ones_bf[:], re,
                                         start=st, stop=sp)
                    if post is not None:
                        post()

                def make_post(psO, psD, p, odd, qp):
                    def post():
                        rec = rp.tile([128, 2, 512], F32, tag="rec")
                        for qs in range(2):
                            nc.vector.reciprocal_approx_fast(
                                rec[:, qs, :], psD[:, qs, :])
                        # normalized outputs into pair-stacked o2 tiles
                        # even h: psO=[or;oi] -> o2r[0:64], o2i[64:128]
                        # odd h:  psO=[oi;or] -> o2i[0:64], o2r[64:128]
                        top, bot = (o2r[p], o2i[p]) if not odd else (o2i[p], o2r[p])
                        for qs in range(2):
                            qsl = slice(1024 * qp + 512 * qs,
                                        1024 * qp + 512 * qs + 512)
                            nc.vector.tensor_mul(
                                top[0:64, qsl], psO[0:64, qs, :], rec[0:64, qs, :])
                            nc.vector.tensor_mul(
                                bot[64:128, qsl], psO[64:128, qs, :],
                                rec[64:128, qs, :])
                    return post

                LAGJ = 5
                for h in range(HPC):
                    p = h // 2
                    odd = h % 2
                    # QN = [qiT ; -qrT] from qt2 via SBUF->SBUF DMA + negate
                    qn = op_.tile([128, N], F32R, tag="qn", bufs=2,
                                  name=f"qn_{h}")
                    nc.sync.dma_start(qn[0:64, :], qt2[h][64:128, :])
                    nc.sync.dma_start(qn[64:128, :], qt2[h][0:64, :])
                    nc.vector.tensor_scalar_mul(qn[64:128, :], qn[64:128, :], -1.0)

                    for qp in range(2):  # 1024-wide query pairs
                        psO = pb.tile([128, 2, 512], F32, tag="psO",
                                      name=f"psO_{h}_{qp}")
                        psD = pb.tile([128, 2, 512], F32, tag="psD",
                                      name=f"psD_{h}_{qp}")
                        post_fn = make_post(psO, psD, p, odd, qp)
                        for kk in range(8):  # kchunk pairs
                            m2b = m2p.tile([128, 2, 1024], F32, tag="m2")
                            for j in range(2):
                                k = 2 * kk + j
                                ksl = slice(128 * k, 128 * k + 128)
                                sr = pb.tile([128, 2, 512], F32, tag="sr", bufs=1,
                                             name=f"sr_{h}_{qp}_{k}")
                                si = pb.tile([128, 2, 512], F32, tag="si", bufs=1,
                                             name=f"si_{h}_{qp}_{k}")
                                for qs in range(2):
                                    qsl = slice(1024 * qp + 512 * qs,
                                                1024 * qp + 512 * qs + 512)
                                    nc.tensor.matmul(
                                        sr[:, qs, :], kt2[h][:, ksl], qt2[h][:, qsl],
                                        start=True, stop=True)
                                    nc.tensor.matmul(
                                        si[:, qs, :], kt2[h][:, ksl], qn[:, qsl],
                                        start=True, stop=True)
                                # DVE reads at most one non-scalar PSUM input,
                                # so m2 = sr^2 + si^2 takes two ops via SBUF.
                                a = app.tile([128, 1024], F32, tag="a")
                                nc.vector._custom_dve(SQ_OP, out=a[:], in0=sr[:, :, :])
                                nc.vector._custom_dve(
                                    SQADD_OP, out=m2b[:, j, :], in0=si[:, :, :],
                                    in1=a[:])
                                if len(pend) > LAGJ:
                                    emit_one()
                            flat = m2b[:, :, :]
                            nc.scalar.activation(flat, flat, AF.Ln)
                            nc.scalar.activation(
                                flat, flat, AF.Exp, bias=lns_t[:, 0:1], scale=0.5)
                            eb = ep.tile([128, 2, 1024], BF16, tag="e")
                            nc.scalar.activation(eb[:, :, :], flat, AF.Exp)
                            for j in range(2):
                                k = 2 * kk + j
                                pend.append(
                                    (psO, psD, vcat[:, k, 128 * h:128 * h + 128],
                                     eb, j, k == 0, k == 15,
                                     post_fn if k == 15 else None))
                while pend:
                    emit_one()

                if dbg:
                    for pi in range(2):
                        nc.sync.dma_start(dbg_o2[pi], o2r[pi][:])
                        nc.sync.dma_start(dbg_o2[2 + pi], o2i[pi][:])
                pb_cm.__exit__(None, None, None)
                rp_cm.__exit__(None, None, None)
                app_cm.__exit__(None, None, None)
                ep_cm.__exit__(None, None, None)
                battn.__exit__(None, None, None)

                # ------------------------------------------ phase C: out-proj
                with (
                    tc.tile_pool(name="wo_p", bufs=1) as wop,
                    tc.tile_pool(name="osb_p", bufs=3) as osb,
                    tc.tile_pool(name="ps_out", bufs=2, space="PSUM") as pc,
                ):
                    wo_sb = {}
                    for ri, dram in [(0, wo_r), (1, wo_i)]:
                        for p in range(2):
                            t = wop.tile([128, DIM], F32R, name=f"wo_{ri}_{p}")
                            nc.sync.dma_start(t[:], dram[p])
                            wo_sb[(ri, p)] = t
                    for ri in range(2):
                        o2 = o2r if ri == 0 else o2i
                        for dc in range(DCH):
                            for ns4 in range(4):
                                nsl = slice(512 * ns4, 512 * ns4 + 512)
                                pso = pc.tile([128, 512], F32, tag="pso",
                                              name=f"pso_{ri}_{dc}_{ns4}")
                                for p in range(2):
                                    nc.tensor.matmul(
                                        pso[:],
                                        wo_sb[(ri, p)][:, 128 * dc:128 * dc + 128],
                                        o2[p][:, nsl], start=(p == 0), stop=(p == 1))
                                ot = osb.tile([128, 512], F32, tag="osb")
                                nc.scalar.activation(
                                    ot[:], pso[:], AF.Identity,
                                    bias=bo_sb[:, dc, ri:ri + 1])
                                nc.sync.dma_start(
                                    out[ri, 128 * dc:128 * dc + 128, nsl], ot[:])

    nc.finalize()
    return nc


_NC_CACHE = None


def _get_program():
    global _NC_CACHE
    if _NC_CACHE is None:
        _NC_CACHE = build_program()
    return _NC_CACHE


# ---------------------------------------------------------------- host wrapper
def _core_inputs(inputs, c):
    b, hg = c // 4, c % 4
    h0 = HPC * hg               # first global head of this core
    r0 = 64 * h0                # first weight row within each of q/k/v blocks
    Wr = np.asarray(inputs["Wqkv_r"], np.float32)
    Wi = np.asarray(inputs["Wqkv_i"], np.float32)
    br = np.asarray(inputs["bqkv_r"], np.float32)
    bi = np.asarray(inputs["bqkv_i"], np.float32)
    Wor = np.asarray(inputs["Wout_r"], np.float32)
    Woi = np.asarray(inputs["Wout_i"], np.float32)
    bor = np.asarray(inputs["bout_r"], np.float32)
    boi = np.asarray(inputs["bout_i"], np.float32)

    def c_(a):
        return np.ascontiguousarray(a, np.float32)

    def blockdiag(Wre, Wim, row0):
        # [2*DIM, 512]: col block h holds [W_re.T head | W_im.T head] stacked
        # on the row (contraction) axis: rows 0:DIM real, DIM:2*DIM imag.
        w = np.zeros((2 * DIM, 512), np.float32)
        for j in range(HPC):
            rr = row0 + 64 * j
            w[0:DIM, 128 * j:128 * j + 64] = Wre[rr:rr + 64, :].T
            w[DIM:2 * DIM, 128 * j + 64:128 * j + 128] = Wim[rr:rr + 64, :].T
        return w

    m = {
        "xr": c_(np.asarray(inputs["x_real"], np.float32)[b].T),
        "xi": c_(np.asarray(inputs["x_imag"], np.float32)[b].T),
        "wq": blockdiag(Wr, Wi, r0),
        "wk": blockdiag(Wr, Wi, DIM + r0),
        "wv_r": c_(Wr[2 * DIM + r0:2 * DIM + r0 + 256, :].T),
        "wv_i": c_(Wi[2 * DIM + r0:2 * DIM + r0 + 256, :].T),
    }
    bq = np.zeros((128, HPC), np.float32)
    bk = np.zeros((128, HPC), np.float32)
    for j in range(HPC):
        rr = r0 + 64 * j
        bq[0:64, j] = br[rr:rr + 64]
        bq[64:128, j] = bi[rr:rr + 64]
        bk[0:64, j] = br[DIM + rr:DIM + rr + 64]
        bk[64:128, j] = bi[DIM + rr:DIM + rr + 64]
    bv = np.zeros((1, 512), np.float32)
    bv[0, 0:256] = br[2 * DIM + r0:2 * DIM + r0 + 256]
    bv[0, 256:512] = bi[2 * DIM + r0:2 * DIM + r0 + 256]
    m["bq"], m["bk"], m["bv"] = bq, bk, bv
    # out-proj pair weights: o2r = [or(h_even); or(h_odd)],
    #                        o2i = [oi(h_odd); oi(h_even)]
    wo_r = np.zeros((2, 128, DIM), np.float32)
    wo_i = np.zeros((2, 128, DIM), np.float32)
    for p in range(2):
        he = r0 + 128 * p        # col offset of h_even's hd block
        ho = he + 64
        wo_r[p, 0:64, :] = Wor[:, he:he + 64].T
        wo_r[p, 64:128, :] = Wor[:, ho:ho + 64].T
        wo_i[p, 0:64, :] = Woi[:, ho:ho + 64].T
        wo_i[p, 64:128, :] = Woi[:, he:he + 64].T
    m["wo_r"], m["wo_i"] = c_(wo_r), c_(wo_i)
    bo = np.zeros((128, DCH, 2), np.float32)
    if hg == 0:  # host sums 4 head-group cores per batch: add bias once
        bo[:, :, 0] = bor.reshape(DCH, 128).T
        bo[:, :, 1] = boi.reshape(DCH, 128).T
    m["bo"] = bo
    m["ones"] = np.ones((128, 128), np.float32)
    return m


def kernel(**inputs):
    nc = _get_program()
    in_maps = [_core_inputs(inputs, c) for c in range(NCORES)]
    res = run_bass_kernel_spmd(nc, in_maps, core_ids=list(range(NCORES)))
    outs = [r_["out"] for r_ in res.results]
    out_r = np.zeros((B, N, DIM), np.float32)
    out_i = np.zeros((B, N, DIM), np.float32)
    for c in range(NCORES):
        b = c // 4
        out_r[b] += outs[c][0].T
        out_i[b] += outs[c][1].T
    return out_r, out_i


if __name__ == "__main__":
    rng = np.random.default_rng(0)
    ins = {
        "x_real": rng.standard_normal((B, N, DIM)).astype(np.float32),
        "x_imag": rng.standard_normal((B, N, DIM)).astype(np.float32),
        "Wqkv_r": (rng.standard_normal((3 * DIM, DIM)) * DIM ** -0.5).astype(np.float32),
        "bqkv_r": (rng.standard_normal(3 * DIM) * 0.01).astype(np.float32),
        "Wqkv_i": (rng.standard_normal((3 * DIM, DIM)) * DIM ** -0.5).astype(np.float32),
        "bqkv_i": (rng.standard_normal(3 * DIM) * 0.01).astype(np.float32),
        "Wout_r": (rng.standard_normal((DIM, DIM)) * DIM ** -0.5).astype(np.float32),
        "bout_r": (rng.standard_normal(DIM) * 0.01).astype(np.float32),
        "Wout_i": (rng.standard_normal((DIM, DIM)) * DIM ** -0.5).astype(np.float32),
        "bout_i": (rng.standard_normal(DIM) * 0.01).astype(np.float32),
    }
    o_r, o_i = kernel(**ins)
    print("ran:", o_r.shape, o_i.shape, o_r[0, 0, :4], o_i[0, 0, :4])

